# Initial kernel scaffold
#
"""Trainium2 Bass kernel for nn_CGLayers (2-layer Clebsch-Gordan GNN).

Sharding: data-parallel over batch B=32 across 8 NeuronCores (4 batches
per core); CG structure matrices and SO3 mixing weights are folded on the
host into fixed "structure matmuls" and replicated to every core.

Optimizations over the first working version:
  * All small constant tensors packed into one HBM tensor (1 DMA instead
    of ~17 Pool-SWDGE DMAs); W1/W3 for both layers each load in one DMA.
  * S-replication (srep) is built by PE broadcast-matmuls against
    host-built selector matrices instead of ~25 SBUF->SBUF DMAs; an
    Activation-engine copy keeps an SBUF version for layer 2.
  * Layer-2 V-replication likewise via PE selector matmuls from the
    layer-1 output tiles (replaces ~20 SBUF->SBUF DMAs).
  * Layer-2 products + W1 matmuls are emitted BEFORE the ops that consume
    the AllGather result, so they overlap the 15us collective instead of
    head-of-line blocking behind it.
  * sqrt/add/reciprocal chain fused to one Rsqrt; m1/m2 neighbor sums
    split per-batch across DVE and Pool.
"""

import math
import numpy as np

MAX_L, C, LAYERS, CUT = 2, 16, 2, 0.5
N_CORES = 8
B_GLOB, N = 32, 128
B_LOC = B_GLOB // N_CORES
BN = B_LOC * N  # 512

# ---------------------------------------------------------------------------
# Clebsch-Gordan coefficients (pure python Racah formula, mirrors reference)
# ---------------------------------------------------------------------------


def _cg_single(j1, m1, j2, m2, j, m):
    if m1 + m2 != m:
        return 0.0
    f = math.factorial
    pre = math.sqrt((2 * j + 1) * f(j + j1 - j2) * f(j - j1 + j2) * f(j1 + j2 - j) / f(j1 + j2 + j + 1))
    pre *= math.sqrt(f(j + m) * f(j - m) * f(j1 - m1) * f(j1 + m1) * f(j2 - m2) * f(j2 + m2))
    s = 0.0
    for k in range(max(0, j2 - j - m1, j1 + m2 - j), min(j1 + j2 - j, j1 - m1, j2 + m2) + 1):
        s += (-1) ** k / (f(k) * f(j1 + j2 - j - k) * f(j1 - m1 - k) * f(j2 + m2 - k) * f(j - j2 + m1 + k) * f(j - j1 - m2 + k))
    return pre * s


PAIRS = {l: [(l1, l2) for l1 in range(MAX_L + 1) for l2 in range(MAX_L + 1)
             if abs(l1 - l2) <= l <= l1 + l2] for l in range(MAX_L + 1)}
CG = {}
for _l in range(MAX_L + 1):
    for _l1, _l2 in PAIRS[_l]:
        _t = np.zeros((2 * _l1 + 1, 2 * _l2 + 1, 2 * _l + 1), np.float32)
        for _i1 in range(2 * _l1 + 1):
            for _i2 in range(2 * _l2 + 1):
                _m = (_i1 - _l1) + (_i2 - _l2)
                if -_l <= _m <= _l:
                    _t[_i1, _i2, _m + _l] = _cg_single(_l1, _i1 - _l1, _l2, _i2 - _l2, _l, _m)
        CG[(_l1, _l2, _l)] = _t

# ---------------------------------------------------------------------------
# Index maps / structure matrices
# ---------------------------------------------------------------------------
IBASE = {0: 0, 1: 1, 2: 4}
LOF = [0, 1, 1, 1, 2, 2, 2, 2, 2]
MOF = [0, 0, 1, 2, 0, 1, 2, 3, 4]


def iof(l, m):
    return IBASE[l] + m


# V / nl feature rows in lambda order (l0, l2, l1) -> 16 + 80 + 48 = 144
VBASE = {0: 0, 2: 16, 1: 96}
NV = 144


def vrow(l, m, c):
    return VBASE[l] + m * 16 + c


def orow(l, k, c):
    return VBASE[l] + k * 16 + c


# P rows: lambda groups in order (l0, l2, l1); within group (j, m, c)
PBASE = {0: 0, 2: 144, 1: 864}
NP = 1296


def prow(lam, j, m, c):
    return PBASE[lam] + j * ((2 * lam + 1) * 16) + m * 16 + c


def _make_chunks():
    chunks = []
    for lam, size in ((0, 144), (2, 720), (1, 432)):
        off = 0
        while off < size:
            n = min(128, size - off)
            chunks.append((lam, PBASE[lam] + off, n))
            off += n
    return chunks


PCHUNKS = _make_chunks()  # [(lam, row0, nrows)] x12

_used = set()
for _l in range(3):
    for _l1, _l2 in PAIRS[_l]:
        _cg = CG[(_l1, _l2, _l)]
        for _m in range(2 * _l1 + 1):
            for _u in range(2 * _l2 + 1):
                if np.abs(_cg[_m, _u]).max() > 0:
                    _a, _b = iof(_l1, _m), iof(_l2, _u)
                    _used.add((min(_a, _b), max(_a, _b)))
STRICT = sorted(p for p in _used if p[0] != p[1])
NZROWS = len(STRICT) * 16        # 512
NW3 = NZROWS + NV                # 656


def build_W1(Wr_layer):
    W1 = np.zeros((NP, NV), np.float32)
    for l in range(3):
        Wr = np.asarray(Wr_layer[l], np.float32)
        for seg, (l1, l2) in enumerate(PAIRS[l]):
            cg = CG[(l1, l2, l)]
            Wr_eff = Wr[seg * 256:(seg + 1) * 256, :].reshape(16, 16, 16).sum(axis=1)
            for m in range(2 * l1 + 1):
                for u in range(2 * l2 + 1):
                    for k in range(2 * l + 1):
                        coeff = cg[m, u, k]
                        if coeff == 0.0:
                            continue
                        r0 = prow(l1, iof(l2, u), m, 0)
                        t0 = orow(l, k, 0)
                        W1[r0:r0 + 16, t0:t0 + 16] += coeff * Wr_eff
    return W1


def build_L2():
    L2 = np.zeros((NV, NZROWS), np.float32)
    for pi, (a, b) in enumerate(STRICT):
        for c in range(16):
            L2[vrow(LOF[a], MOF[a], c), pi * 16 + c] = 1.0
            L2[vrow(LOF[b], MOF[b], c), pi * 16 + c] = 1.0
    return L2


def build_W3(Wn_layer):
    W3 = np.zeros((NW3, NV), np.float32)
    pair_index = {p: idx for idx, p in enumerate(STRICT)}
    for l in range(3):
        Wn = np.asarray(Wn_layer[l], np.float32)
        for seg, (l1, l2) in enumerate(PAIRS[l]):
            cg = CG[(l1, l2, l)]
            Wn_eff = Wn[seg * 16:(seg + 1) * 16, :]
            for m in range(2 * l1 + 1):
                for u in range(2 * l2 + 1):
                    for k in range(2 * l + 1):
                        coeff = cg[m, u, k]
                        if coeff == 0.0:
                            continue
                        a, b = iof(l1, m), iof(l2, u)
                        t0 = orow(l, k, 0)
                        for c in range(16):
                            if a == b:
                                W3[NZROWS + vrow(LOF[a], MOF[a], c), t0:t0 + 16] += coeff * Wn_eff[c]
                            else:
                                pi = pair_index[(min(a, b), max(a, b))]
                                W3[pi * 16 + c, t0:t0 + 16] += 0.5 * coeff * Wn_eff[c]
                                W3[NZROWS + vrow(LOF[a], MOF[a], c), t0:t0 + 16] -= 0.5 * coeff * Wn_eff[c]
                                W3[NZROWS + vrow(LOF[b], MOF[b], c), t0:t0 + 16] -= 0.5 * coeff * Wn_eff[c]
    return W3


def build_SELK():
    # [144, 64]: cols 16:64 = sum over k of squares, l-order (l0,l1,l2)
    S = np.zeros((NV, 64), np.float32)
    for l in range(3):
        for k in range(2 * l + 1):
            for c in range(16):
                S[orow(l, k, c), 16 + l * 16 + c] = 1.0
    return S


def build_SELS0():
    # [16, 64]: cols 0:16 pick the nl l0 block (s0 rows)
    S = np.zeros((16, 64), np.float32)
    for c in range(16):
        S[c, c] = 1.0
    return S


def build_SEL3():
    S = np.zeros((NV, 3), np.float32)
    for l in range(3):
        for k in range(2 * l + 1):
            for c in range(16):
                S[orow(l, k, c), l] = 1.0
    return S


def build_BC():
    # cols [0:128) bcast s[0]; [128:256) s[1]; [256:384) s[2]
    BC = np.zeros((3, 384), np.float32)
    BC[0, 0:128] = 1.0
    BC[1, 128:256] = 1.0
    BC[2, 256:384] = 1.0
    return BC


def build_BC112():
    # vec64 = BC112a @ s1 + BC112b @ s1sq:
    # rows 0:16 -> s1[0]; rows 16:64 -> s1sq l-pattern
    A = np.zeros((3, 64), np.float32)
    Bm = np.zeros((3, 64), np.float32)
    A[0, 0:16] = 1.0
    for l in range(3):
        Bm[l, 16 + 16 * l:16 + 16 * (l + 1)] = 1.0
    return A, Bm


def build_E9():
    # srep selector: srep_chunk = E9_ci^T @ S  ([9,nr]^T [9,BN] -> [nr,BN])
    E = np.zeros((16, 12 * 128), np.float32)
    for ci, (lam, r0, nr) in enumerate(PCHUNKS):
        run = (2 * lam + 1) * 16
        for p in range(nr):
            j = (r0 + p - PBASE[lam]) // run
            E[j, ci * 128 + p] = 1.0
    return E


def build_VS():
    # vrep selector: vrep_chunk = VSA_ci^T @ vbig + VSB_ci^T @ vB
    A = np.zeros((128, 12 * 128), np.float32)
    Bm = np.zeros((16, 12 * 128), np.float32)
    nz = []
    for ci, (lam, r0, nr) in enumerate(PCHUNKS):
        run = (2 * lam + 1) * 16
        nzb = False
        for p in range(nr):
            off = (r0 + p - PBASE[lam]) % run
            vr = VBASE[lam] + off
            if vr < 128:
                A[vr, ci * 128 + p] = 1.0
            else:
                Bm[vr - 128, ci * 128 + p] = 1.0
                nzb = True
        nz.append(nzb)
    return A, Bm, nz


E9M = build_E9()
VSA, VSB, VSB_NZ = build_VS()

# pack1 column offsets (f32, [128, PACK1_COLS])
IDENT_O = 0
L2A_O = 128
L2B_O = 640
SELKA_O = 1152
SELKB_O = 1216
SELS0_O = 1280
SEL3A_O = 1344
SEL3B_O = 1347
BC_O = 1350
BCA_O = 1734
BCB_O = 1798
CVEC_O = 1862
E9_O = 1864
PACK1_COLS = E9_O + 12 * 128

# pack2: [128, 3072]: VSA at 0, VSB (rows 0:16) at 1536
VSB_O = 1536
PACK2_COLS = 3072

# lambda -> number of V rows
LAMROWS = {0: 16, 1: 48, 2: 80}

# ---------------------------------------------------------------------------
# Bass program
# ---------------------------------------------------------------------------
_BUILT = None
SPLIT_WAITS = True


def _drainpatch():
    """walrus in this container rejects sem waits on Drain instructions
    ('Too many sync wait commands'); carry the tail waits on SP nops."""
    import concourse.tile as tile
    from bass_rust import ScopedClock, VectorClock

    def _patched(self, tick_clock, wait_clock):
        gc_vec = tick_clock.global_clock
        nproc = len(gc_vec)
        for proc in range(nproc):
            t = gc_vec[proc]
            if t > 0:
                vec = [0] * nproc
                vec[proc] = t
                sub = ScopedClock({None: VectorClock(vec)})
                d = self.nc.sync.nop(hint="tail_wait", nofuse=True)
                wait_clock.add_sem_waits(d.ins, sub)
        self.nc.sync.drain()
        self.nc.all_engine_barrier()
        assert self.sems is not None
        popped = self.nc._tile_sem_poison_stack.pop()
        assert popped is self._sem_poison
        # sem reset ISA op miscompiles on this walrus; single-shot kernel
        # relies on NRT re-initializing semaphores per execution.
        self.nc._state.prepend_free_semaphores(
            [s.num for s in self.sems.allocated().values()])
        self.nc.all_engine_barrier()

    tile.TileContext._drain_and_barrier = _patched


def build_program():
    import contextlib
    import concourse.bass as bass
    import concourse.mybir as mybir
    import concourse.tile as tile

    _drainpatch()
    dt = mybir.dt
    f32 = dt.float32
    f32r = dt.float32r
    AF = mybir.ActivationFunctionType
    ALU = mybir.AluOpType
    AX = mybir.AxisListType

    nc = bass.Bass("TRN2", target_bir_lowering=False, debug=False,
                   num_devices=N_CORES)

    dp = nc.declare_dram_parameter
    vrep0_d = dp("vrep0", [12, 128, BN], f32, isOutput=False)
    rp_d = dp("rp", [B_LOC, N, N, 3], f32, isOutput=False)
    norms_d = dp("norms", [B_LOC, N, N], f32, isOutput=False)
    pack1_d = dp("pack1", [128, PACK1_COLS], f32r, isOutput=False)
    pack2_d = dp("pack2", [128, PACK2_COLS], f32r, isOutput=False)
    w1_d = dp("w1", [LAYERS, 12 * 128, NV], f32r, isOutput=False)
    w3_d = dp("w3", [LAYERS, 6 * 128, NV], f32r, isOutput=False)
    out_d = dp("out", [B_LOC, N, 128], f32, isOutput=True)
    ss2_d = dp("ss2", [1, 3], f32, isOutput=True)

    W3CH = [(0, 128), (128, 128), (256, 128), (384, 128), (512, 128), (640, 16)]

    with tile.TileContext(nc) as tc:
        with contextlib.ExitStack() as ctx:
            const = ctx.enter_context(tc.tile_pool(name="const", bufs=1))
            work = ctx.enter_context(tc.tile_pool(name="work", bufs=1))
            prod = ctx.enter_context(tc.tile_pool(name="prod", bufs=8))
            vload = ctx.enter_context(tc.tile_pool(name="vload", bufs=2))
            psum = ctx.enter_context(tc.tile_pool(name="psum", bufs=6, space="PSUM"))
            pst = ctx.enter_context(tc.tile_pool(name="pst", bufs=2, space="PSUM"))
            dram = ctx.enter_context(tc.tile_pool(name="dram", bufs=1, space="DRAM"))

            # ---------------- input loads (node-major) ----------------
            rp = work.tile([N, B_LOC, N, 3], f32, name="rp", tag="rp")
            nc.sync.dma_start(rp[:], rp_d[:].rearrange("b i j c -> i b j c"))
            nnode = work.tile([N, B_LOC, N], f32, name="nnode", tag="nnode")
            nc.sync.dma_start(nnode[:], norms_d[:].rearrange("b i j -> i b j"))

            # ---------------- constant loads (few, packed) ----------------
            pk1 = const.tile([128, PACK1_COLS], f32r, name="pk1", tag="pk1")
            nc.gpsimd.dma_start(pk1[:], pack1_d[:])
            w1all = const.tile([128, 2 * 12, NV], f32r, name="w1all", tag="w1all")
            nc.gpsimd.dma_start(
                w1all[:], w1_d[:].rearrange("l (ci p) f -> p (l ci) f", p=128))
            w3all = const.tile([128, 2 * 6, NV], f32r, name="w3all", tag="w3all")
            nc.scalar.dma_start(
                w3all[:], w3_d[:].rearrange("l (ci p) f -> p (l ci) f", p=128))
            # layer-1 V-replication, host-prepacked, 4 group loads of 3 chunks
            vrt = []
            for g in range(4):
                t = vload.tile([128, 3, BN], f32, name=f"vrt{g}", tag="vrt")
                nc.sync.dma_start(
                    t[:], vrep0_d[g * 3:(g + 1) * 3].rearrange("c p f -> p c f"))
                vrt.append(t)
            # pack2 (layer-2 V-replication selectors) reuses the vload pool
            # space once the vrep0 groups are consumed
            pk2a = vload.tile([128, 3, BN], f32r, name="pk2a", tag="vrt")
            nc.scalar.dma_start(pk2a[:].rearrange("p c f -> p (c f)"),
                                pack2_d[:, 0:VSB_O])
            pk2b = vload.tile([128, 3, BN], f32r, name="pk2b", tag="vrt")
            nc.scalar.dma_start(pk2b[:].rearrange("p c f -> p (c f)"),
                                pack2_d[:, VSB_O:PACK2_COLS])

            def pk2a_sl(ci, rows, nr):
                return pk2a[0:rows, ci // 4, (ci % 4) * 128:(ci % 4) * 128 + nr]

            def pk2b_sl(ci, rows, nr):
                return pk2b[0:rows, ci // 4, (ci % 4) * 128:(ci % 4) * 128 + nr]

            ident = pk1[:, IDENT_O:IDENT_O + 128].bitcast(f32)

            # ---------------- sph phase ----------------
            pp = work.tile([N, 6, B_LOC, N], f32, name="pp", tag="pp")
            x = rp[:, :, :, 0]
            y = rp[:, :, :, 1]
            z = rp[:, :, :, 2]
            sx = work.tile([N, B_LOC, N], f32, name="sx", tag="sx")
            sy = work.tile([N, B_LOC, N], f32, name="sy", tag="sy")
            sz = work.tile([N, B_LOC, N], f32, name="sz", tag="sz")
            nc.scalar.activation(sx[:], x, AF.Square)
            nc.scalar.activation(sy[:], y, AF.Square)
            nc.scalar.activation(sz[:], z, AF.Square)
            r2a = work.tile([N, B_LOC, N], f32, name="r2a", tag="r2a")
            nc.vector.tensor_add(r2a[:], sx[:], sy[:])
            r2 = work.tile([N, B_LOC, N], f32, name="r2", tag="r2")
            nc.vector.tensor_add(r2[:], r2a[:], sz[:])
            # raw pair products start immediately, in parallel with squares
            rxy = work.tile([N, B_LOC, N], f32, name="rxy", tag="rxy")
            ryz = work.tile([N, B_LOC, N], f32, name="ryz", tag="ryz")
            rxz = work.tile([N, B_LOC, N], f32, name="rxz", tag="rxz")
            nc.vector.tensor_mul(rxy[:], x, y)
            nc.gpsimd.tensor_mul(ryz[:], y, z)
            nc.gpsimd.tensor_mul(rxz[:], x, z)
            ir2 = work.tile([N, B_LOC, N], f32, name="ir2", tag="ir2")
            nc.vector.reciprocal(ir2[:], r2[:])
            rinv = work.tile([N, B_LOC, N], f32, name="rinv", tag="rinv")
            nc.scalar.activation(rinv[:], ir2[:], AF.Sqrt)
            xyzr = work.tile([N, 3, B_LOC, N], f32, name="xyzr", tag="xyzr")
            nc.vector.tensor_mul(xyzr[:, 0], x, rinv[:])
            nc.vector.tensor_mul(xyzr[:, 1], y, rinv[:])
            nc.gpsimd.tensor_mul(xyzr[:, 2], z, rinv[:])
            # quadratic fields = raw products * (1/r^2); no sqrt dependency
            nc.vector.tensor_mul(pp[:, 0], rxy[:], ir2[:])
            nc.gpsimd.tensor_mul(pp[:, 1], ryz[:], ir2[:])
            nc.vector.tensor_mul(pp[:, 2], rxz[:], ir2[:])
            nc.vector.tensor_mul(pp[:, 3], sx[:], ir2[:])
            nc.gpsimd.tensor_mul(pp[:, 4], sy[:], ir2[:])
            nc.vector.tensor_mul(pp[:, 5], sz[:], ir2[:])
            m1 = work.tile([N, 3, B_LOC], f32, name="m1", tag="m1")
            nc.vector.reduce_sum(m1[:], xyzr[:], axis=AX.X)
            m2 = work.tile([N, 6, B_LOC], f32, name="m2", tag="m2")
            nc.vector.reduce_sum(m2[:], pp[:], axis=AX.X)

            snode = work.tile([N, B_LOC, 16], f32, name="snode", tag="snode")
            nc.vector.memset(snode[:], 0.0)
            nc.vector.memset(snode[:, :, 0], float(N * 0.28209479))
            for comp in range(3):  # Y1 order (y,z,x)
                src = m1[:, [1, 2, 0][comp], :]
                nc.gpsimd.tensor_scalar_mul(snode[:, :, 1 + comp], src, 0.48860251)
            nc.gpsimd.tensor_scalar_mul(snode[:, :, 4], m2[:, 0, :], 1.09254843)
            nc.gpsimd.tensor_scalar_mul(snode[:, :, 5], m2[:, 1, :], 1.09254843)
            nc.vector.tensor_scalar(snode[:, :, 6], m2[:, 5, :],
                                    3.0 * 0.31539157, float(-N * 0.31539157),
                                    op0=ALU.mult, op1=ALU.add)
            nc.gpsimd.tensor_scalar_mul(snode[:, :, 7], m2[:, 2, :], 1.09254843)
            d8 = work.tile([N, B_LOC], f32, name="d8", tag="d8")
            nc.gpsimd.tensor_sub(d8[:], m2[:, 3, :], m2[:, 4, :])
            nc.gpsimd.tensor_scalar_mul(snode[:, :, 8], d8[:], 0.54627422)

            # ---------------- transposes: S, conn ----------------
            spsum = pst.tile([16, B_LOC, N], f32, name="spsum", tag="pst")
            for b in range(B_LOC):
                nc.tensor.transpose(spsum[:, b, :], snode[:, b, :], ident)
            sfm = work.tile([16, BN], f32, name="sfm", tag="sfm")
            nc.scalar.copy(sfm[:].bitcast(f32r), spsum[:].rearrange("j b n -> j (b n)"))

            ctp = pst.tile([N, B_LOC, N], f32, name="ctp", tag="pst")
            for b in range(B_LOC):
                nc.tensor.transpose(ctp[:, b, :], nnode[:, b, :], ident)
            connT = work.tile([N, B_LOC, N], f32, name="connT", tag="connT")
            nc.vector.tensor_scalar(connT[:], ctp[:], CUT, None, op0=ALU.is_lt)

            srepT = []
            vecs = {}

            def emit_products(ly, vsrc, split):
                accA, accB, groups = {}, {}, {}
                for ci, (lam, r0, nr) in enumerate(PCHUNKS):
                    groups.setdefault(lam if split else 0, []).append(ci)
                for g in groups:
                    accA[g] = psum.tile([128, BN], f32, name=f"accA{ly}{g}", tag="ps")
                    accB[g] = psum.tile([16, BN], f32, name=f"accB{ly}{g}", tag="ps")
                pend = None
                for g, cis in sorted(groups.items()):
                    for idx, ci in enumerate(cis):
                        lam, r0, nr = PCHUNKS[ci]
                        if ly == 0:
                            # srep chunk via PE broadcast matmul from S
                            sps = pst.tile([nr, BN], f32, name=f"sps{ci}", tag="pst")
                            nc.tensor.matmul(
                                sps[:],
                                pk1[0:16, E9_O + ci * 128:E9_O + ci * 128 + nr].bitcast(f32r),
                                sfm[:].bitcast(f32r), start=True, stop=True)
                            st = const.tile([nr, BN], f32, name=f"srepT{ci}", tag=f"srepT{ci}")
                            nc.scalar.copy(st[:], sps[:])
                            srepT.append(st)
                            vr_ap = vrt[ci // 3][0:nr, ci % 3, :]
                            srep_ap = sps[:]
                        else:
                            vps = pst.tile([nr, BN], f32, name=f"vps{ci}", tag="pst")
                            vA, vB = vsrc
                            nc.tensor.matmul(
                                vps[:], pk2a_sl(ci, 128, nr),
                                vA[:].bitcast(f32r),
                                start=True, stop=not VSB_NZ[ci])
                            if VSB_NZ[ci]:
                                nc.tensor.matmul(
                                    vps[:], pk2b_sl(ci, 16, nr),
                                    vB[:].bitcast(f32r),
                                    start=False, stop=True)
                            vr_ap = vps[:]
                            srep_ap = srepT[ci][:]
                        pt = prod.tile([nr, BN], f32, name=f"pt{ly}_{ci}", tag="pchunk")
                        if ly == 0 and ci % 3 == 2:
                            # Pool cannot read PSUM: use the SBUF srep copy
                            nc.gpsimd.tensor_mul(pt[:].bitcast(f32r), vr_ap, srepT[ci][:])
                        else:
                            nc.vector.tensor_mul(pt[:].bitcast(f32r), vr_ap, srep_ap)
                        # software pipeline: defer W1 matmuls by one chunk so
                        # PE can start the next replication matmul instead of
                        # head-of-line blocking on this chunk's DVE product
                        if pend is not None:
                            pg, pci, pnr, ppt, pfirst, plast = pend
                            nc.tensor.matmul(accA[pg][:],
                                             w1all[0:pnr, ly * 12 + pci, 0:128],
                                             ppt[:].bitcast(f32r),
                                             start=pfirst, stop=plast)
                            nc.tensor.matmul(accB[pg][:],
                                             w1all[0:pnr, ly * 12 + pci, 128:144],
                                             ppt[:].bitcast(f32r),
                                             start=pfirst, stop=plast)
                        pend = (g, ci, nr, pt, idx == 0, idx == len(cis) - 1)
                pg, pci, pnr, ppt, pfirst, plast = pend
                nc.tensor.matmul(accA[pg][:],
                                 w1all[0:pnr, ly * 12 + pci, 0:128],
                                 ppt[:].bitcast(f32r),
                                 start=pfirst, stop=plast)
                nc.tensor.matmul(accB[pg][:],
                                 w1all[0:pnr, ly * 12 + pci, 128:144],
                                 ppt[:].bitcast(f32r),
                                 start=pfirst, stop=plast)
                return accA, accB

            def emit_post(ly, accA, accB, split):
                relA = work.tile([128, BN], f32, name=f"relA_{ly}", tag=f"relA_{ly}")
                relB = work.tile([16, BN], f32, name=f"relB_{ly}", tag=f"relB_{ly}")
                if not split:
                    nc.vector.tensor_copy(relA[:], accA[0][:])
                    nc.scalar.copy(relB[:], accB[0][:])
                else:
                    t1 = work.tile([128, BN], f32, name="cmb1", tag="cmb1")
                    nc.vector.tensor_scalar_mul(t1[:], accA[0][:], vecs[0][0:128, :])
                    t2 = work.tile([128, BN], f32, name="cmb2", tag="cmb2")
                    nc.vector.scalar_tensor_tensor(
                        t2[:], accA[2][:], vecs[2][0:128, :], t1[:],
                        op0=ALU.mult, op1=ALU.add)
                    nc.vector.scalar_tensor_tensor(
                        relA[:], accA[1][:], vecs[1][0:128, :], t2[:],
                        op0=ALU.mult, op1=ALU.add)
                    t1b = work.tile([16, BN], f32, name="cmb1b", tag="cmb1b")
                    nc.vector.tensor_scalar_mul(t1b[:], accB[0][:], vecs[0][0:16, :])
                    t2b = work.tile([16, BN], f32, name="cmb2b", tag="cmb2b")
                    nc.vector.scalar_tensor_tensor(
                        t2b[:], accB[2][:], vecs[2][0:16, :], t1b[:],
                        op0=ALU.mult, op1=ALU.add)
                    nc.vector.scalar_tensor_tensor(
                        relB[:], accB[1][:], vecs[1][0:16, :], t2b[:],
                        op0=ALU.mult, op1=ALU.add)

                # transpose rel -> [j, f] per b; message passing
                mpPSA = psum.tile([128, B_LOC, N], f32, name=f"mpA{ly}", tag="ps")
                mpPSB = psum.tile([16, B_LOC, N], f32, name=f"mpB{ly}", tag="ps")
                for b in range(B_LOC):
                    rtp = pst.tile([N, NV], f32, name=f"rtp{ly}{b}", tag="pst")
                    nc.tensor.transpose(rtp[:, 0:128], relA[:, b * N:(b + 1) * N], ident)
                    nc.tensor.transpose(rtp[:, 128:144], relB[:, b * N:(b + 1) * N],
                                        ident[0:16, 0:16])
                    relT = work.tile([N, NV], f32, name=f"relT{ly}{b}", tag="relT")
                    nc.scalar.copy(relT[:], rtp[:])
                    nc.tensor.matmul(mpPSA[:, b, :], relT[:, 0:128],
                                     connT[:, b, :], start=True, stop=True)
                    nc.tensor.matmul(mpPSB[:, b, :], relT[:, 128:144],
                                     connT[:, b, :], start=True, stop=True)
                mpF = work.tile([128, BN], f32, name=f"mpF_{ly}", tag=f"mpF_{ly}")
                mpB = work.tile([16, BN], f32, name=f"mpB_{ly}", tag=f"mpB_{ly}")
                nc.vector.tensor_copy(mpF[:].bitcast(f32r), mpPSA[:].rearrange("f b n -> f (b n)"))
                nc.vector.tensor_copy(mpB[:].bitcast(f32r), mpPSB[:].rearrange("f b n -> f (b n)"))
                sqmpA = work.tile([128, BN], f32, name=f"sqmA_{ly}", tag=f"sqmA_{ly}")
                sqmpB = work.tile([16, BN], f32, name=f"sqmB_{ly}", tag=f"sqmB_{ly}")
                nc.scalar.activation(sqmpA[:].bitcast(f32r), mpPSA[:].rearrange("f b n -> f (b n)"), AF.Square)
                nc.scalar.activation(sqmpB[:].bitcast(f32r), mpPSB[:].rearrange("f b n -> f (b n)"), AF.Square)

                sqz = []
                for zc in range(4):
                    zps = psum.tile([128, BN], f32, name=f"zps{ly}{zc}", tag="ps")
                    nc.tensor.matmul(zps[:], pk1[:, L2A_O + zc * 128:L2A_O + (zc + 1) * 128].bitcast(f32r),
                                     mpF[:].bitcast(f32r), start=True, stop=False)
                    nc.tensor.matmul(zps[:], pk1[0:16, L2B_O + zc * 128:L2B_O + (zc + 1) * 128].bitcast(f32r),
                                     mpB[:].bitcast(f32r), start=False, stop=True)
                    sq = work.tile([128, BN], f32, name=f"sqz{ly}{zc}", tag="sqz")
                    nc.scalar.activation(sq[:].bitcast(f32r), zps[:], AF.Square)
                    sqz.append(sq)

                nlA = psum.tile([128, BN], f32, name=f"nlA{ly}", tag="ps")
                nlB = psum.tile([16, BN], f32, name=f"nlB{ly}", tag="ps")
                rhs = sqz + [sqmpA, sqmpB]
                for ci in range(6):
                    nr3 = W3CH[ci][1]
                    nc.tensor.matmul(nlA[:], w3all[0:nr3, ly * 6 + ci, 0:128],
                                     rhs[ci][:].bitcast(f32r),
                                     start=(ci == 0), stop=(ci == 5))
                for ci in range(6):
                    nr3 = W3CH[ci][1]
                    nc.tensor.matmul(nlB[:], w3all[0:nr3, ly * 6 + ci, 128:144],
                                     rhs[ci][:].bitcast(f32r),
                                     start=(ci == 0), stop=(ci == 5))
                vbig = work.tile([128, BN], f32, name=f"v{ly + 1}big", tag=f"v{ly + 1}big")
                vB = work.tile([16, BN], f32, name=f"v{ly + 1}B", tag=f"v{ly + 1}B")
                nc.vector.tensor_copy(vbig[:].bitcast(f32r), nlA[:])
                nc.vector.tensor_copy(vB[:].bitcast(f32r), nlB[:])
                sqnlA = work.tile([128, BN], f32, name=f"sqnA_{ly}", tag=f"sqnA_{ly}")
                sqnlB = work.tile([16, BN], f32, name=f"sqnB_{ly}", tag=f"sqnB_{ly}")
                colA = work.tile([128, 1], f32, name=f"colA{ly}", tag="colA")
                colB = work.tile([16, 1], f32, name=f"colB{ly}", tag="colB")
                nc.scalar.activation(sqnlA[:].bitcast(f32r), nlA[:], AF.Square, accum_out=colA[:])
                nc.scalar.activation(sqnlB[:].bitcast(f32r), nlB[:], AF.Square, accum_out=colB[:])
                ssp = pst.tile([3, 1], f32, name=f"ssp{ly}", tag="pst")
                nc.tensor.matmul(ssp[:], pk1[:, SEL3A_O:SEL3A_O + 3].bitcast(f32), colA[:], start=True, stop=False)
                nc.tensor.matmul(ssp[:], pk1[0:16, SEL3B_O:SEL3B_O + 3].bitcast(f32), colB[:], start=False, stop=True)
                ssl = work.tile([3, 1], f32, name=f"ssl_{ly}", tag=f"ssl_{ly}")
                nc.scalar.copy(ssl[:], ssp[:])
                return vbig, vB, sqnlA, sqnlB, ssl

            # ===== layer 1 =====
            acc1A, acc1B = emit_products(0, None, split=False)
            (v1big, v1B, sqn1A, sqn1B, ssl1) = emit_post(0, acc1A, acc1B, split=False)
            # op1 psum [112, BN]: cols 0:16 = raw s0 rows, 64:112 = raw sn sums
            op1 = pst.tile([64, BN], f32, name="op1", tag="pst")
            nc.tensor.matmul(op1[:], pk1[0:16, SELS0_O:SELS0_O + 64], v1big[0:16, :].bitcast(f32r), start=True, stop=False)
            nc.tensor.matmul(op1[:], pk1[:, SELKA_O:SELKA_O + 64], sqn1A[:].bitcast(f32r), start=False, stop=False)
            nc.tensor.matmul(op1[:], pk1[0:16, SELKB_O:SELKB_O + 64], sqn1B[:].bitcast(f32r), start=False, stop=True)
            # free the PSUM slot right away: op1 is consumed only after the
            # collective, and the layer-2 vps tiles cycle through this pool
            op1s = work.tile([64, BN], f32, name="op1s", tag="op1s")
            nc.scalar.copy(op1s[:], op1[:])

            # collective: AllGather of layer-1 SS partials
            cc_in = dram.tile([1, 3], f32)
            cc_out = dram.tile([N_CORES, 3], f32, addr_space="Shared")
            nc.sync.dma_start(cc_in[0, :], ssl1[:, 0])
            nc.gpsimd.collective_compute(
                "AllGather", mybir.AluOpType.bypass,
                ins=[cc_in.opt()], outs=[cc_out.opt()],
                replica_groups=[list(range(N_CORES))])

            # ===== layer 2 head: products + W1 matmuls =====
            # (independent of the collective result; overlaps its 15us latency)
            acc2A, acc2B = emit_products(1, (v1big, v1B), split=True)

            # collective readback + normalization scales
            ssall = work.tile([3, N_CORES], f32, name="ssall", tag="ssall")
            nc.sync.dma_start(ssall[:], cc_out[:].rearrange("c l -> l c"))
            ssg = work.tile([3, 1], f32, name="ssg", tag="ssg")
            nc.vector.reduce_sum(ssg[:], ssall[:], axis=AX.X)
            irs = work.tile([3, 1], f32, name="irs", tag="irs")
            nc.vector.reciprocal(irs[:], ssg[:])
            rts = work.tile([3, 1], f32, name="rts", tag="rts")
            nc.scalar.activation(rts[:], irs[:], AF.Sqrt)
            s1 = work.tile([3, 1], f32, name="s1", tag="s1")
            nc.vector.tensor_scalar_mul(s1[:], rts[:], pk1[0:3, CVEC_O:CVEC_O + 1].bitcast(f32))
            s1sq = work.tile([3, 1], f32, name="s1sq", tag="s1sq")
            nc.vector.tensor_mul(s1sq[:], s1[:], s1[:])
            for lam in range(3):
                vp = pst.tile([128, 1], f32, name=f"vp{lam}", tag="pst")
                nc.tensor.matmul(vp[:], pk1[0:3, BC_O + lam * 128:BC_O + (lam + 1) * 128].bitcast(f32), s1[:],
                                 start=True, stop=True)
                vt = work.tile([128, 1], f32, name=f"vec_{lam}", tag=f"vec_{lam}")
                nc.scalar.copy(vt[:], vp[:])
                vecs[lam] = vt
            v112p = pst.tile([64, 1], f32, name="v112p", tag="pst")
            nc.tensor.matmul(v112p[:], pk1[0:3, BCA_O:BCA_O + 64].bitcast(f32), s1[:], start=True, stop=False)
            nc.tensor.matmul(v112p[:], pk1[0:3, BCB_O:BCB_O + 64].bitcast(f32), s1sq[:], start=False, stop=True)
            vec112 = work.tile([64, 1], f32, name="vec112", tag="vec112")
            nc.scalar.copy(vec112[:], v112p[:])

            # layer-1 output block: scale raw s0/sn rows by [s1[0] | s1sq]
            outl1 = work.tile([64, BN], f32, name="outl1", tag="outl1")
            nc.vector.tensor_scalar_mul(outl1[:], op1s[:], vec112[:])

            # ===== layer 2 tail =====
            (v2big, v2B, sqn2A, sqn2B, ssl2) = emit_post(1, acc2A, acc2B, split=True)
            nc.sync.dma_start(ss2_d[0, :], ssl2[:, 0])
            # raw layer-2 rows: cols 64:80 = s0 rows (from nl2), 80:128 = sn sums
            op2 = pst.tile([64, BN], f32, name="op2", tag="pst")
            nc.tensor.matmul(op2[:], pk1[0:16, SELS0_O:SELS0_O + 64], v2big[0:16, :].bitcast(f32r), start=True, stop=False)
            nc.tensor.matmul(op2[:], pk1[:, SELKA_O:SELKA_O + 64], sqn2A[:].bitcast(f32r), start=False, stop=False)
            nc.tensor.matmul(op2[:], pk1[0:16, SELKB_O:SELKB_O + 64], sqn2B[:].bitcast(f32r), start=False, stop=True)
            outl2 = work.tile([64, BN], f32, name="outl2", tag="outl2")
            nc.vector.tensor_copy(outl2[:], op2[:])

            # transpose both 112-row blocks to node-major and store
            onm = work.tile([N, B_LOC, 128], f32, name="onm", tag="onm")
            for li, osrc in ((0, outl1), (1, outl2)):
                otp = pst.tile([N, B_LOC, 64], f32, name=f"otp_{li}", tag="pst")
                for b in range(B_LOC):
                    nc.tensor.transpose(otp[:, b, :], osrc[:, b * N:(b + 1) * N],
                                        ident[0:64, 0:64])
                nc.scalar.copy(onm[:, :, li * 64:(li + 1) * 64], otp[:])
            o_ap = out_d[:].rearrange("b n f -> n b f")
            nc.sync.dma_start(o_ap[:], onm[:])

    if SPLIT_WAITS:
        _split_waits(nc, mybir)
    return nc


def _split_waits(nc, mybir, maxw=1):
    """This container's walrus rejects instructions carrying more than one
    semaphore wait; spill extra waits onto same-engine NoOps placed just
    before the instruction."""
    k = [0]
    for f in nc.m.functions:
        for bb in f.blocks:
            newl = []
            for ins in bb.instructions:
                si = ins.sync_info
                if si is not None and len(si.on_wait) > maxw:
                    waits = list(si.on_wait)
                    for w in waits[:-maxw]:
                        k[0] += 1
                        nop = mybir.InstDrain(name=f"wsplit_{k[0]}", ins=[], outs=[])
                        nop.engine = ins.engine
                        nop.sync_info = mybir.SyncInfo(on_wait=[w], on_update=[])
                        newl.append(nop)
                    ins.sync_info = mybir.SyncInfo(on_wait=waits[-maxw:],
                                                  on_update=list(si.on_update))
                newl.append(ins)
            bb.instructions = newl


def _get_program():
    global _BUILT
    if _BUILT is None:
        _BUILT = build_program()
    return _BUILT


def _make_const_inputs(Wr, Wn):
    def _pad_chunks(M, chunks):
        out = np.zeros((len(chunks) * 128, M.shape[1]), np.float32)
        for ci, (r0, nr) in enumerate(chunks):
            out[ci * 128:ci * 128 + nr] = M[r0:r0 + nr]
        return out
    w1c = [(r0, nr) for (_lam, r0, nr) in PCHUNKS]
    w3c = [(0, 128), (128, 128), (256, 128), (384, 128), (512, 128), (640, 16)]
    w1 = np.stack([_pad_chunks(build_W1({l: np.asarray(Wr[l][ly]) for l in range(3)}), w1c)
                   for ly in range(LAYERS)])
    w3 = np.stack([_pad_chunks(build_W3({l: np.asarray(Wn[l][ly]) for l in range(3)}), w3c)
                   for ly in range(LAYERS)])
    bc112a, bc112b = build_BC112()
    L2 = build_L2()
    selk = build_SELK()
    sel3 = build_SEL3()
    pack1 = np.zeros((128, PACK1_COLS), np.float32)
    pack1[:, IDENT_O:IDENT_O + 128] = np.eye(128, dtype=np.float32)
    pack1[0:128, L2A_O:L2A_O + 512] = L2[0:128]
    pack1[0:16, L2B_O:L2B_O + 512] = L2[128:144]
    pack1[0:128, SELKA_O:SELKA_O + 64] = selk[0:128]
    pack1[0:16, SELKB_O:SELKB_O + 64] = selk[128:144]
    pack1[0:16, SELS0_O:SELS0_O + 64] = build_SELS0()
    pack1[0:128, SEL3A_O:SEL3A_O + 3] = sel3[0:128]
    pack1[0:16, SEL3B_O:SEL3B_O + 3] = sel3[128:144]
    pack1[0:3, BC_O:BC_O + 384] = build_BC()
    pack1[0:3, BCA_O:BCA_O + 64] = bc112a
    pack1[0:3, BCB_O:BCB_O + 64] = bc112b
    pack1[0:3, CVEC_O:CVEC_O + 1] = (C / np.array([[1.0], [3.0], [5.0]], np.float32))
    pack1[0:16, E9_O:E9_O + 12 * 128] = E9M
    pack2 = np.zeros((128, PACK2_COLS), np.float32)
    pack2[0:128, 0:12 * 128] = VSA
    pack2[0:16, VSB_O:VSB_O + 12 * 128] = VSB
    return {
        "pack1": pack1,
        "pack2": pack2,
        "w1": w1.astype(np.float32),
        "w3": w3.astype(np.float32),
    }


def kernel(v0, v1, v2, rel_pos, norms, Wr0, Wr1, Wr2, Wn0, Wn1, Wn2):
    v0, v1, v2 = np.asarray(v0), np.asarray(v1), np.asarray(v2)
    rel_pos, norms = np.asarray(rel_pos), np.asarray(norms)
    consts = _make_const_inputs({0: Wr0, 1: Wr1, 2: Wr2},
                                {0: Wn0, 1: Wn1, 2: Wn2})
    vfm = np.concatenate([v0.reshape(B_GLOB * N, 16),
                          v2.reshape(B_GLOB * N, 80),
                          v1.reshape(B_GLOB * N, 48)], axis=-1).T  # [144, B*N]
    vmap = np.empty(NP, np.int64)
    for lam in (0, 2, 1):
        for j in range(9):
            for m in range(2 * lam + 1):
                for c in range(16):
                    vmap[prow(lam, j, m, c)] = VBASE[lam] + m * 16 + c
    vrep_full = vfm[vmap]  # [1296, B*N]
    in_maps = []
    for cc in range(N_CORES):
        sl = slice(cc * B_LOC, (cc + 1) * B_LOC)
        m = dict(consts)
        vr = np.zeros((12, 128, BN), np.float32)
        vcore = vrep_full[:, cc * B_LOC * N:(cc + 1) * B_LOC * N]
        for ci, (_lam, r0, nr) in enumerate(PCHUNKS):
            vr[ci, 0:nr] = vcore[r0:r0 + nr]
        m["vrep0"] = vr
        m["rp"] = np.ascontiguousarray(rel_pos[sl])
        m["norms"] = np.ascontiguousarray(norms[sl])
        in_maps.append(m)

    from concourse.bass_utils import run_bass_kernel_spmd
    nc = _get_program()
    res = run_bass_kernel_spmd(nc, in_maps, list(range(N_CORES)))
    outs = [res.results[cc]["out"] for cc in range(N_CORES)]
    ss2 = sum(res.results[cc]["ss2"][0] for cc in range(N_CORES))
    out = np.concatenate(outs, axis=0)  # [32, 128, 128]
    scale2 = C / (np.array([1.0, 3.0, 5.0], np.float32) * np.sqrt(ss2))
    fin = np.ones(128, np.float32)
    fin[64:80] = scale2[0]
    fin[80:96] = scale2[0] ** 2
    fin[96:112] = scale2[1] ** 2
    fin[112:128] = scale2[2] ** 2
    return (out * fin[None, None, :]).astype(np.float32)



# revision 123
# speedup vs baseline: 1.3542x; 1.3542x over previous
"""Trainium2 Bass kernel for nn_CGLayers (2-layer Clebsch-Gordan GNN).

Sharding: data-parallel over batch B=32 across 8 NeuronCores (4 batches
per core); CG structure matrices and SO3 mixing weights are folded on the
host into fixed "structure matmuls" and replicated to every core.

v2 restructure over the 98us baseline:
  * All V-replication is built on-PE from a small [144, BN] feature tile
    via selector matmuls (no 3.15MB host-replicated vrep DMA).
  * Stationary matmul operands (W1/W3/selectors) are bf16: halves their
    DMA bytes at no PE cost (cost model keys on the moving operand).
  * sph phase: zz moment eliminated (sum x^2+y^2+z^2 = N), moment sums
    feed a single per-batch PE matmul that builds the spherical-harmonic
    sums directly (no snode assembly / transposes).
  * Layer-1 normalization scale is folded into 3 scaled copies of the
    connectivity matrix, so the per-group combine of layer-2 message
    passing happens inside PE PSUM accumulation.  Everything in layer 2
    except the (scale-dependent) message-pass + CG-square + W3 runs
    during the 15us AllGather; both layers' outputs leave the kernel
    unnormalized and the host applies the per-l scales (extends the
    baseline's existing host-side `fin` scaling).
"""

import math
import numpy as np

MAX_L, C, LAYERS, CUT = 2, 16, 2, 0.5
N_CORES = 8
B_GLOB, N = 32, 128
B_LOC = B_GLOB // N_CORES
BN = B_LOC * N  # 512

# ---------------------------------------------------------------------------
# Clebsch-Gordan coefficients (pure python Racah formula, mirrors reference)
# ---------------------------------------------------------------------------


def _cg_single(j1, m1, j2, m2, j, m):
    if m1 + m2 != m:
        return 0.0
    f = math.factorial
    pre = math.sqrt((2 * j + 1) * f(j + j1 - j2) * f(j - j1 + j2) * f(j1 + j2 - j) / f(j1 + j2 + j + 1))
    pre *= math.sqrt(f(j + m) * f(j - m) * f(j1 - m1) * f(j1 + m1) * f(j2 - m2) * f(j2 + m2))
    s = 0.0
    for k in range(max(0, j2 - j - m1, j1 + m2 - j), min(j1 + j2 - j, j1 - m1, j2 + m2) + 1):
        s += (-1) ** k / (f(k) * f(j1 + j2 - j - k) * f(j1 - m1 - k) * f(j2 + m2 - k) * f(j - j2 + m1 + k) * f(j - j1 - m2 + k))
    return pre * s


PAIRS = {l: [(l1, l2) for l1 in range(MAX_L + 1) for l2 in range(MAX_L + 1)
             if abs(l1 - l2) <= l <= l1 + l2] for l in range(MAX_L + 1)}
CG = {}
for _l in range(MAX_L + 1):
    for _l1, _l2 in PAIRS[_l]:
        _t = np.zeros((2 * _l1 + 1, 2 * _l2 + 1, 2 * _l + 1), np.float32)
        for _i1 in range(2 * _l1 + 1):
            for _i2 in range(2 * _l2 + 1):
                _m = (_i1 - _l1) + (_i2 - _l2)
                if -_l <= _m <= _l:
                    _t[_i1, _i2, _m + _l] = _cg_single(_l1, _i1 - _l1, _l2, _i2 - _l2, _l, _m)
        CG[(_l1, _l2, _l)] = _t

# ---------------------------------------------------------------------------
# Index maps / structure matrices
# ---------------------------------------------------------------------------
IBASE = {0: 0, 1: 1, 2: 4}
LOF = [0, 1, 1, 1, 2, 2, 2, 2, 2]
MOF = [0, 0, 1, 2, 0, 1, 2, 3, 4]


def iof(l, m):
    return IBASE[l] + m


# V / nl feature rows in lambda order (l0, l2, l1) -> 16 + 80 + 48 = 144
VBASE = {0: 0, 2: 16, 1: 96}
NV = 144


def vrow(l, m, c):
    return VBASE[l] + m * 16 + c


def orow(l, k, c):
    return VBASE[l] + k * 16 + c


# P rows: lambda groups in order (l0, l2, l1); within group (j, m, c).
# j=0 (the Y0 spherical row) is an exact constant N*0.28209479, so those
# product rows are a LINEAR term folded into W1LIN on the host; P keeps
# only j=1..8 -> 128/640/384 rows per group, 9 full 128-row chunks.
PBASE = {0: 0, 2: 128, 1: 768}
NP = 1152
Y0SUM = N * 0.28209479


def prow(lam, j, m, c):
    # j in 1..8
    return PBASE[lam] + (j - 1) * ((2 * lam + 1) * 16) + m * 16 + c


def _make_chunks():
    chunks = []
    for lam, size in ((0, 128), (2, 640), (1, 384)):
        off = 0
        while off < size:
            n = min(128, size - off)
            chunks.append((lam, PBASE[lam] + off, n))
            off += n
    return chunks


PCHUNKS = _make_chunks()  # [(lam, row0, nrows)] x9
GROUP_LAMS = (0, 2, 1)     # chunk-group order (matches PCHUNKS order)
BAND = {0: 0, 2: 1, 1: 2}  # lam -> 16-row band index in packed B accs

_used = set()
for _l in range(3):
    for _l1, _l2 in PAIRS[_l]:
        _cg = CG[(_l1, _l2, _l)]
        for _m in range(2 * _l1 + 1):
            for _u in range(2 * _l2 + 1):
                if np.abs(_cg[_m, _u]).max() > 0:
                    _a, _b = iof(_l1, _m), iof(_l2, _u)
                    _used.add((min(_a, _b), max(_a, _b)))
STRICT = sorted(p for p in _used if p[0] != p[1])
NZROWS = len(STRICT) * 16        # 512
NW3 = NZROWS + NV                # 656


def build_W1(Wr_layer):
    """Returns (W1 over j>=1 P-rows, W1LIN [NV, NV] for the j=0 rows)."""
    W1 = np.zeros((NP, NV), np.float32)
    W1L = np.zeros((NV, NV), np.float32)
    for l in range(3):
        Wr = np.asarray(Wr_layer[l], np.float32)
        for seg, (l1, l2) in enumerate(PAIRS[l]):
            cg = CG[(l1, l2, l)]
            Wr_eff = Wr[seg * 256:(seg + 1) * 256, :].reshape(16, 16, 16).sum(axis=1)
            for m in range(2 * l1 + 1):
                for u in range(2 * l2 + 1):
                    for k in range(2 * l + 1):
                        coeff = cg[m, u, k]
                        if coeff == 0.0:
                            continue
                        t0 = orow(l, k, 0)
                        j = iof(l2, u)
                        if j == 0:
                            v0 = vrow(l1, m, 0)
                            W1L[v0:v0 + 16, t0:t0 + 16] += coeff * Y0SUM * Wr_eff
                        else:
                            r0 = prow(l1, j, m, 0)
                            W1[r0:r0 + 16, t0:t0 + 16] += coeff * Wr_eff
    return W1, W1L


def build_L2():
    L2 = np.zeros((NV, NZROWS), np.float32)
    for pi, (a, b) in enumerate(STRICT):
        for c in range(16):
            L2[vrow(LOF[a], MOF[a], c), pi * 16 + c] = 1.0
            L2[vrow(LOF[b], MOF[b], c), pi * 16 + c] = 1.0
    return L2


def build_W3(Wn_layer):
    W3 = np.zeros((NW3, NV), np.float32)
    pair_index = {p: idx for idx, p in enumerate(STRICT)}
    for l in range(3):
        Wn = np.asarray(Wn_layer[l], np.float32)
        for seg, (l1, l2) in enumerate(PAIRS[l]):
            cg = CG[(l1, l2, l)]
            Wn_eff = Wn[seg * 16:(seg + 1) * 16, :]
            for m in range(2 * l1 + 1):
                for u in range(2 * l2 + 1):
                    for k in range(2 * l + 1):
                        coeff = cg[m, u, k]
                        if coeff == 0.0:
                            continue
                        a, b = iof(l1, m), iof(l2, u)
                        t0 = orow(l, k, 0)
                        for c in range(16):
                            if a == b:
                                W3[NZROWS + vrow(LOF[a], MOF[a], c), t0:t0 + 16] += coeff * Wn_eff[c]
                            else:
                                pi = pair_index[(min(a, b), max(a, b))]
                                W3[pi * 16 + c, t0:t0 + 16] += 0.5 * coeff * Wn_eff[c]
                                W3[NZROWS + vrow(LOF[a], MOF[a], c), t0:t0 + 16] -= 0.5 * coeff * Wn_eff[c]
                                W3[NZROWS + vrow(LOF[b], MOF[b], c), t0:t0 + 16] -= 0.5 * coeff * Wn_eff[c]
    return W3


def build_SELK():
    # [144, 64]: cols 16:64 = sum over k of squares, l-order (l0,l1,l2)
    S = np.zeros((NV, 64), np.float32)
    for l in range(3):
        for k in range(2 * l + 1):
            for c in range(16):
                S[orow(l, k, c), 16 + l * 16 + c] = 1.0
    return S


def build_SELS0():
    # [16, 64]: cols 0:16 pick the nl l0 block (s0 rows)
    S = np.zeros((16, 64), np.float32)
    for c in range(16):
        S[c, c] = 1.0
    return S


def build_SEL3():
    S = np.zeros((NV, 3), np.float32)
    for l in range(3):
        for k in range(2 * l + 1):
            for c in range(16):
                S[orow(l, k, c), l] = 1.0
    return S


def build_BC():
    # cols [0:128) bcast s[0]; [128:256) s[1]; [256:384) s[2]
    BC = np.zeros((3, 384), np.float32)
    BC[0, 0:128] = 1.0
    BC[1, 128:256] = 1.0
    BC[2, 256:384] = 1.0
    return BC


def build_E9():
    # srep selector: srep_chunk = E9_ci^T @ sfm  ([16,nr]^T [16,BN] -> [nr,BN])
    E = np.zeros((16, 12 * 128), np.float32)
    for ci, (lam, r0, nr) in enumerate(PCHUNKS):
        run = (2 * lam + 1) * 16
        for p in range(nr):
            j = 1 + (r0 + p - PBASE[lam]) // run
            E[j, ci * 128 + p] = 1.0
    return E


def build_VS():
    # vrep selector: vrep_chunk = VSA_ci^T @ vA + VSB_ci^T @ vB
    A = np.zeros((128, 12 * 128), np.float32)
    Bm = np.zeros((16, 12 * 128), np.float32)
    nz = []
    for ci, (lam, r0, nr) in enumerate(PCHUNKS):
        run = (2 * lam + 1) * 16
        nzb = False
        for p in range(nr):
            off = (r0 + p - PBASE[lam]) % run
            vr = VBASE[lam] + off
            if vr < 128:
                A[vr, ci * 128 + p] = 1.0
            else:
                Bm[vr - 128, ci * 128 + p] = 1.0
                nzb = True
        nz.append(nzb)
    return A, Bm, nz


def build_MMAT():
    # sfm[m, x] = sum_s MMAT[s, m] * meas[s, x]; meas slots:
    # 0=ones, 1=mx, 2=my, 3=mz, 4=mxy, 5=mxx, 6=myy, 7=myz, 8=mxz
    # (mzz eliminated: mxx+myy+mzz = N exactly)
    M = np.zeros((9, 16), np.float32)
    M[0, 0] = N * 0.28209479
    M[0, 6] = 0.31539157 * 2.0 * N
    M[1, 3] = 0.48860251   # Y1 order (y,z,x)
    M[2, 1] = 0.48860251
    M[3, 2] = 0.48860251
    M[4, 4] = 1.09254843
    M[5, 6] = -3.0 * 0.31539157
    M[5, 8] = 0.54627422
    M[6, 6] = -3.0 * 0.31539157
    M[6, 8] = -0.54627422
    M[7, 5] = 1.09254843
    M[8, 7] = 1.09254843
    return M


E9M = build_E9()
VSA, VSB, VSB_NZ = build_VS()

# pk128f (f32) column offsets
IDENT_O = 0
BC_O = 128
CVEC2_O = 512
SEL3AF_O = 513
SEL3BF_O = 516
F_COLS = 519
# pk128b (bf16) column offsets
IDENTB_O = 0
L2A_O = 128
SELKA_O = 640
SEL3A_O = 704
MMAT_O = 707
B_COLS = 723
# pkSb (bf16, 16 rows) column offsets
E9_O = 0
L2B_O = 1536
SELKB_O = 2048
SELS0_O = 2112
SEL3B_O = 2176
S_COLS = 2179

W3CH = [(0, 128), (128, 128), (256, 128), (384, 128), (512, 128), (640, 16)]

# ---------------------------------------------------------------------------
# Bass program
# ---------------------------------------------------------------------------
_BUILT = None
SPLIT_WAITS = True


def _drainpatch():
    """walrus in this container rejects sem waits on Drain instructions
    ('Too many sync wait commands'); carry the tail waits on SP nops."""
    import concourse.tile as tile
    from bass_rust import ScopedClock, VectorClock

    def _patched(self, tick_clock, wait_clock):
        gc_vec = tick_clock.global_clock
        nproc = len(gc_vec)
        for proc in range(nproc):
            t = gc_vec[proc]
            if t > 0:
                vec = [0] * nproc
                vec[proc] = t
                sub = ScopedClock({None: VectorClock(vec)})
                d = self.nc.sync.nop(hint="tail_wait", nofuse=True)
                wait_clock.add_sem_waits(d.ins, sub)
        self.nc.sync.drain()
        assert self.sems is not None
        popped = self.nc._tile_sem_poison_stack.pop()
        assert popped is self._sem_poison
        # sem reset ISA op miscompiles on this walrus; single-shot kernel
        # relies on NRT re-initializing semaphores per execution.
        self.nc._state.prepend_free_semaphores(
            [s.num for s in self.sems.allocated().values()])

    tile.TileContext._drain_and_barrier = _patched


def build_program():
    import contextlib
    import concourse.bass as bass
    import concourse.mybir as mybir
    import concourse.tile as tile

    _drainpatch()
    dt = mybir.dt
    f32 = dt.float32
    f32r = dt.float32r
    bf16 = dt.bfloat16
    AF = mybir.ActivationFunctionType
    ALU = mybir.AluOpType
    AX = mybir.AxisListType

    nc = bass.Bass("TRN2", target_bir_lowering=False, debug=False,
                   num_devices=N_CORES)

    dp = nc.declare_dram_parameter
    rp_d = dp("rp", [B_LOC, N, N, 3], bf16, isOutput=False)
    norms_d = dp("norms", [B_LOC, N, N], f32, isOutput=False)
    vfa_d = dp("vfa", [128, BN], bf16, isOutput=False)
    vfb_d = dp("vfb", [16, BN], bf16, isOutput=False)
    pkf_d = dp("pkf", [128, F_COLS], f32r, isOutput=False)
    pkb_d = dp("pkb", [128, B_COLS], bf16, isOutput=False)
    pks_d = dp("pks", [16, S_COLS], bf16, isOutput=False)
    vsa_d = dp("vsa", [128, 9 * 128], bf16, isOutput=False)
    vsb_d = dp("vsb", [16, 9 * 128], bf16, isOutput=False)
    w1_d = dp("w1", [128, LAYERS * 9, NV], bf16, isOutput=False)
    w3_d = dp("w3", [128, LAYERS * 6, NV], bf16, isOutput=False)
    # j=0 linear terms: slot 0 = layer-1 combined, slots 1..3 = layer-2
    # masked per v-row group (GROUP_LAMS order)
    linm_d = dp("linm", [128, 4, NV], bf16, isOutput=False)
    linb_d = dp("linb", [16, 4, NV], bf16, isOutput=False)
    out_d = dp("out", [2, 64, BN], bf16, isOutput=True)
    ss1_d = dp("ss1", [1, 3], f32, isOutput=True)
    ss2_d = dp("ss2", [1, 3], f32, isOutput=True)

    with tile.TileContext(nc) as tc:
        with contextlib.ExitStack() as ctx:
            const = ctx.enter_context(tc.tile_pool(name="const", bufs=1))
            work = ctx.enter_context(tc.tile_pool(name="work", bufs=1))
            prod = ctx.enter_context(tc.tile_pool(name="prod", bufs=6))
            psum = ctx.enter_context(tc.tile_pool(name="psum", bufs=1, space="PSUM"))
            dram = ctx.enter_context(tc.tile_pool(name="dram", bufs=1, space="DRAM"))

            # ---------------- input / constant loads ----------------
            # sync (SP HWDGE) queue, priority order
            rp = work.tile([N, B_LOC, N, 3], bf16, name="rp", tag="rp")
            nc.sync.dma_start(rp[:], rp_d[:].rearrange("b i j c -> i b j c"))
            pks = const.tile([16, S_COLS], bf16, name="pks", tag="pks")
            nc.sync.dma_start(pks[:], pks_d[:])
            pkf = const.tile([128, F_COLS], f32r, name="pkf", tag="pkf")
            nc.sync.dma_start(pkf[:], pkf_d[:])
            pkb = const.tile([128, B_COLS], bf16, name="pkb", tag="pkb")
            nc.sync.dma_start(pkb[:], pkb_d[:])
            vfa = const.tile([128, BN], bf16, name="vfa", tag="vfa")
            nc.sync.dma_start(vfa[:], vfa_d[:])
            vfb = const.tile([16, BN], bf16, name="vfb", tag="vfb")
            nc.sync.dma_start(vfb[:], vfb_d[:])
            nnode = work.tile([N, B_LOC, N], f32, name="nnode", tag="nnode")
            nc.sync.dma_start(nnode[:], norms_d[:].rearrange("b i j -> i b j"))
            # weight/selector tiles after the small packs on the same queue:
            # the shared DMA engine serves in issue order, so the packs that
            # gate the first PE work land first
            vsa = const.tile([128, 9, 128], bf16, name="vsa", tag="vsa")
            nc.sync.dma_start(vsa[:].rearrange("p c f -> p (c f)"), vsa_d[:])
            vsb = const.tile([16, 9, 128], bf16, name="vsb", tag="vsb")
            nc.sync.dma_start(vsb[:].rearrange("p c f -> p (c f)"), vsb_d[:])
            w1all = const.tile([128, 2 * 9, NV], bf16, name="w1all", tag="w1all")
            nc.sync.dma_start(w1all[:], w1_d[:])
            w3all = const.tile([128, 2 * 6, NV], bf16, name="w3all", tag="w3all")
            nc.sync.dma_start(w3all[:], w3_d[:])
            linm = const.tile([128, 4, NV], bf16, name="linm", tag="linm")
            nc.sync.dma_start(linm[:], linm_d[:])
            linb = const.tile([16, 4, NV], bf16, name="linb", tag="linb")
            nc.sync.dma_start(linb[:], linb_d[:])

            identf = pkf[:, IDENT_O:IDENT_O + 128].bitcast(f32)
            identb = pkb[:, IDENTB_O:IDENTB_O + 128]

            # ---------------- sph phase ----------------
            x = rp[:, :, :, 0]
            y = rp[:, :, :, 1]
            z = rp[:, :, :, 2]
            sx = work.tile([N, B_LOC, N], bf16, name="sx", tag="sx")
            sy = work.tile([N, B_LOC, N], bf16, name="sy", tag="sy")
            sz = work.tile([N, B_LOC, N], bf16, name="sz", tag="sz")
            nc.scalar.activation(sx[:], x, AF.Square)
            nc.vector.tensor_mul(sy[:], y, y)
            nc.scalar.activation(sz[:], z, AF.Square)
            r2a = work.tile([N, B_LOC, N], bf16, name="r2a", tag="r2a")
            nc.vector.tensor_add(r2a[:], sx[:], sy[:])
            r2 = work.tile([N, B_LOC, N], bf16, name="r2", tag="r2")
            nc.vector.tensor_add(r2[:], r2a[:], sz[:])
            ir2 = work.tile([N, B_LOC, N], f32, name="ir2", tag="ir2")
            nc.vector.reciprocal(ir2[:], r2[:])
            rinv = work.tile([N, B_LOC, N], f32, name="rinv", tag="rinv")
            nc.scalar.activation(rinv[:], ir2[:], AF.Sqrt)

            # moment fields -> per-(i,b) sums; meas slots per build_MMAT
            def pp_tile(shape, dtyp, name):
                return psum.tile(shape, dtyp, name=name, tag="pp", bufs=4)

            # unit-vector fields f = (x,y,z)/r; all second-order fields are
            # products/squares of those (spread over Act/Pool), so the raw
            # pair products and their 1/r^2 muls disappear entirely.
            # meas slots: 0=ones, 1=mx,2=my,3=mz,4=mxy, 5=mxx,6=myy, 7=myz,8=mxz
            meas = work.tile([N, B_LOC, 9], bf16, name="meas", tag="meas")
            nc.vector.memset(meas[:, :, 0], 1.0)
            fldd = work.tile([N, B_LOC, 4, N], bf16, name="fldd", tag="fldd")
            flda = work.tile([N, B_LOC, 2, N], bf16, name="flda", tag="flda")
            fldp = work.tile([N, B_LOC, 2, N], bf16, name="fldp", tag="fldp")
            fx, fy, fz = (fldd[:, :, 0, :], fldd[:, :, 1, :], fldd[:, :, 2, :])
            nc.vector.tensor_mul(fx, x, rinv[:])
            nc.vector.tensor_mul(fy, y, rinv[:])
            nc.gpsimd.tensor_mul(fz, z, rinv[:])
            nc.vector.tensor_mul(fldd[:, :, 3, :], fx, fy)        # xy/r^2
            nc.scalar.activation(flda[:, :, 0, :], fx, AF.Square)  # xx/r^2
            nc.scalar.activation(flda[:, :, 1, :], fy, AF.Square)  # yy/r^2
            nc.gpsimd.tensor_mul(fldp[:, :, 0, :], fy, fz)        # yz/r^2
            nc.gpsimd.tensor_mul(fldp[:, :, 1, :], fx, fz)        # xz/r^2
            with nc.allow_low_precision("bf16 moment sums feed bf16 sfm matmul"):
                nc.vector.reduce_sum(meas[:, :, 1:5], fldd[:], axis=AX.X)
                nc.vector.reduce_sum(meas[:, :, 5:7], flda[:], axis=AX.X)
                nc.vector.reduce_sum(meas[:, :, 7:9], fldp[:], axis=AX.X)

            # meas -> sfm via per-b transpose + MMAT matmul
            measT = pp_tile([9, B_LOC, N], bf16, "measT")
            for b in range(B_LOC):
                nc.tensor.transpose(measT[:, b, :], meas[:, b, :], identb)
            measS = work.tile([9, B_LOC, N], bf16, name="measS", tag="measS")
            nc.vector.tensor_copy(measS[:], measT[:])
            sfm_ps = pp_tile([16, B_LOC, N], f32, "sfm_ps")
            mmat = pkb[0:9, MMAT_O:MMAT_O + 16]
            nc.tensor.matmul(sfm_ps[:].rearrange("m b n -> m (b n)"), mmat,
                             measS[:].rearrange("s b n -> s (b n)"),
                             start=True, stop=True)
            sfm = work.tile([16, BN], bf16, name="sfm", tag="sfm")
            nc.scalar.copy(sfm[:], sfm_ps[:].rearrange("m b n -> m (b n)"))

            # connectivity (transposed): connT[j, b, i] = norms[b,i,j] < CUT


            ctp = pp_tile([N, B_LOC, N], f32, "ctp")
            for b in range(B_LOC):
                nc.tensor.transpose(ctp[:, b, :], nnode[:, b, :], identf)
            connT = work.tile([N, B_LOC, N], bf16, name="connT", tag="connT")
            nc.vector.tensor_scalar(connT[:], ctp[:], CUT, None, op0=ALU.is_lt)

            POOL_CIS = {1, 3, 5, 7}   # product chunks multiplied on Pool
            srepT = {}                 # ci -> SBUF copy of srep chunk (L1-built)

            def emit_products(ly, vA_ap, vB_ap, accA_tiles, accB_tile, split,
                              post_group=None, alloc_accA=None):
                """srep/vrep selector matmuls + elementwise product + W1.
                split=True: accA_tiles dict lam -> psum [128, BN], accB_tile
                [48, BN] with per-lam 16-row bands.  split=False: single
                accA tile + accB rows 0:16, one accumulation over all 12
                chunks.  Engines can read only one PSUM operand, so srep is
                staged to SBUF (layer 1 builds srepT, layer 2 reuses it)."""
                glam = {}
                for ci, (lam, r0, nr) in enumerate(PCHUNKS):
                    glam.setdefault(lam, []).append(ci)
                nchunks = len(PCHUNKS)

                def _emit_w1(lam, ci, nr, pt, first, last):
                    # psum out base partition must be 0/32/64/96 -> 32-spaced
                    bnd = BAND[lam] * 32 if split else 0
                    nc.tensor.matmul(accA_tiles[lam][:],
                                     w1all[0:nr, ly * 9 + ci, 0:128],
                                     pt[:], start=first, stop=last)
                    nc.tensor.matmul(accB_tile[bnd:bnd + 16, :],
                                     w1all[0:nr, ly * 9 + ci, 128:144],
                                     pt[:], start=first, stop=last)
                    if last and post_group is not None:
                        post_group(lam)

                def _emit_lin(lam, slot):
                    # j=0 linear term opens each accumulation group
                    bnd = BAND[lam] * 32 if split else 0
                    nc.tensor.matmul(accA_tiles[lam][:], linm[0:128, slot, 0:128],
                                     vA_ap, start=True, stop=False)
                    nc.tensor.matmul(accA_tiles[lam][:], linb[0:16, slot, 0:128],
                                     vB_ap, start=False, stop=False)
                    nc.tensor.matmul(accB_tile[bnd:bnd + 16, :],
                                     linm[0:128, slot, 128:144],
                                     vA_ap, start=True, stop=False)
                    nc.tensor.matmul(accB_tile[bnd:bnd + 16, :],
                                     linb[0:16, slot, 128:144],
                                     vB_ap, start=False, stop=False)

                pend = []
                gidx = 0
                for gi, lam in enumerate(GROUP_LAMS):
                    cis = glam[lam]
                    if alloc_accA is not None:
                        alloc_accA(lam)
                    if split:
                        _emit_lin(lam, 1 + gi)
                    elif gi == 0:
                        _emit_lin(lam, 0)
                    for idx, ci in enumerate(cis):
                        _, r0, nr = PCHUNKS[ci]
                        if ly == 0:
                            sps = pp_tile([nr, BN], f32, f"sps{ly}_{ci}")
                            nc.tensor.matmul(
                                sps[:], pks[:, E9_O + ci * 128:E9_O + ci * 128 + nr],
                                sfm[:], start=True, stop=True)
                            st = work.tile([nr, BN], f32, name=f"srepT{ci}",
                                           tag=f"srepT{ci}")
                            if ci % 2 == 0:
                                nc.scalar.copy(st[:], sps[:])
                            else:
                                nc.vector.tensor_copy(st[:], sps[:])
                            srepT[ci] = st
                        vps = pp_tile([nr, BN], f32, f"vps{ly}_{ci}")
                        nc.tensor.matmul(
                            vps[:], vsa[0:128, ci, 0:nr], vA_ap,
                            start=True, stop=not VSB_NZ[ci])
                        if VSB_NZ[ci]:
                            nc.tensor.matmul(
                                vps[:], vsb[0:16, ci, 0:nr], vB_ap,
                                start=False, stop=True)
                        pt = prod.tile([nr, BN], bf16, name=f"pt{ly}_{ci}",
                                       tag="pchunk")
                        if ci in POOL_CIS:
                            # Pool cannot read PSUM at all: stage vps too
                            vpsS = prod.tile([nr, BN], f32, name=f"vpsS{ly}_{ci}",
                                             tag="vpsS", bufs=2)
                            nc.scalar.copy(vpsS[:], vps[:])
                            nc.gpsimd.tensor_mul(pt[:], vpsS[:], srepT[ci][:])
                        else:
                            nc.vector.tensor_mul(pt[:], vps[:], srepT[ci][:])
                        # lin matmuls opened the accumulation: never start here
                        if split:
                            first, last = False, idx == len(cis) - 1
                        else:
                            first, last = False, gidx == nchunks - 1
                        # software pipeline: defer W1 matmuls by two chunks so
                        # PE stays ahead of the DVE product muls
                        if len(pend) == 5:
                            _emit_w1(*pend.pop(0))
                        pend.append((lam, ci, nr, pt, first, last))
                        gidx += 1
                for p in pend:
                    _emit_w1(*p)

            def emit_quad(ly, mpA_ps, mpB_ps, mpF, mpB):
                """z chunks + squares + W3 -> returns nl psum pair."""
                sqz = []
                for zc in range(4):
                    zps = pp_tile([128, BN], f32, f"zps{ly}{zc}")
                    nc.tensor.matmul(zps[:], pkb[:, L2A_O + zc * 128:L2A_O + (zc + 1) * 128],
                                     mpF[:], start=True, stop=False)
                    nc.tensor.matmul(zps[:], pks[:, L2B_O + zc * 128:L2B_O + (zc + 1) * 128],
                                     mpB[:], start=False, stop=True)
                    sq = work.tile([128, BN], bf16, name=f"sqz{ly}{zc}", tag="sqz",
                                   bufs=4)
                    if zc % 2 == 0:
                        nc.scalar.activation(sq[:], zps[:], AF.Square)
                    else:
                        # engines may read only one PSUM operand: stage + mul
                        zS = work.tile([128, BN], bf16, name=f"zS{ly}{zc}",
                                       tag="zS", bufs=2)
                        nc.vector.tensor_copy(zS[:], zps[:])
                        nc.gpsimd.tensor_mul(sq[:], zS[:], zS[:])
                    sqz.append(sq)
                sqmpA = work.tile([128, BN], bf16, name=f"sqmA_{ly}", tag=f"sqmA_{ly}")
                sqmpB = work.tile([16, BN], bf16, name=f"sqmB_{ly}", tag=f"sqmB_{ly}")
                nc.scalar.activation(sqmpA[:], mpA_ps[:].rearrange("f b n -> f (b n)"), AF.Square)
                nc.gpsimd.tensor_mul(sqmpB[:], mpB[:], mpB[:])
                nlA = psum.tile([128, BN], f32, name=f"nlA{ly}", tag="A", bufs=3)
                nlB = psum.tile([16, BN], f32, name=f"nlB{ly}", tag="B", bufs=1)
                rhs = sqz + [sqmpA, sqmpB]
                # accumulate sqmp chunks first: they are ready right after mp,
                # before the z-square chunks stream in
                order = [4, 5, 0, 1, 2, 3]
                for k, ci in enumerate(order):
                    nr3 = W3CH[ci][1]
                    nc.tensor.matmul(nlA[:], w3all[0:nr3, ly * 6 + ci, 0:128],
                                     rhs[ci][:], start=(k == 0), stop=(k == 5))
                for k, ci in enumerate(order):
                    nr3 = W3CH[ci][1]
                    nc.tensor.matmul(nlB[:], w3all[0:nr3, ly * 6 + ci, 128:144],
                                     rhs[ci][:], start=(k == 0), stop=(k == 5))
                return nlA, nlB

            # ===================== layer 1 =====================
            acc1A = psum.tile([128, BN], f32, name="acc1A", tag="A", bufs=3)
            acc1B = psum.tile([16, BN], f32, name="acc1B", tag="B", bufs=1)
            emit_products(0, vfa[:], vfb[:],
                          {0: acc1A, 2: acc1A, 1: acc1A}, acc1B, split=False)

            # ===================== layer-1 rel -> mp =====================
            relA1 = work.tile([128, BN], bf16, name="relA1", tag="relA1")
            nc.vector.tensor_copy(relA1[:], acc1A[:])
            relB1 = work.tile([16, BN], bf16, name="relB1", tag="relB1")
            nc.scalar.copy(relB1[:], acc1B[:])
            mpA1 = psum.tile([128, B_LOC, N], f32, name="mpA1", tag="A", bufs=3)
            mpB1 = psum.tile([16, B_LOC, N], f32, name="mpB1", tag="B", bufs=1)
            # all transposes first, then copies, then matmuls: keeps PE
            # streaming instead of ping-ponging with the copy engines per b
            relT1 = {}
            for b in range(B_LOC):
                rtp = pp_tile([N, NV], bf16, f"rtp1{b}")
                nc.tensor.transpose(rtp[:, 0:128], relA1[:, b * N:(b + 1) * N], identb)
                nc.tensor.transpose(rtp[:, 128:144], relB1[0:16, b * N:(b + 1) * N],
                                    identb[0:16, 0:16])
                relT = work.tile([N, NV], bf16, name=f"relT1{b}", tag=f"relT1{b}")
                if b % 2 == 0:
                    nc.vector.tensor_copy(relT[:], rtp[:])
                else:
                    nc.scalar.copy(relT[:], rtp[:])
                relT1[b] = relT
            for b in range(B_LOC):
                nc.tensor.matmul(mpA1[:, b, :], relT1[b][:, 0:128], connT[:, b, :],
                                 start=True, stop=True)
            for b in range(B_LOC):
                nc.tensor.matmul(mpB1[:, b, :], relT1[b][:, 128:144], connT[:, b, :],
                                 start=True, stop=True)
            mpF1 = work.tile([128, BN], bf16, name="mpF1", tag="mpF1")
            nc.vector.tensor_copy(mpF1[:], mpA1[:].rearrange("f b n -> f (b n)"))
            mpBs1 = work.tile([16, BN], bf16, name="mpBs1", tag="mpBs1")
            nc.scalar.copy(mpBs1[:], mpB1[:].rearrange("f b n -> f (b n)"))

            nl1A, nl1B = emit_quad(0, mpA1, mpB1, mpF1, mpBs1)

            # layer-1 outputs (raw; host applies norm scales)
            v1big = work.tile([128, BN], bf16, name="v1big", tag="v1big")
            nc.vector.tensor_copy(v1big[:], nl1A[:])
            v1B = work.tile([16, BN], bf16, name="v1B", tag="v1B")
            nc.vector.tensor_copy(v1B[:], nl1B[:])
            sqn1A = work.tile([128, BN], bf16, name="sqn1A", tag="sqn1A")
            sqn1B = work.tile([16, BN], bf16, name="sqn1B", tag="sqn1B")
            colA1 = work.tile([128, 1], f32, name="colA1", tag="colA")
            colB1 = work.tile([16, 1], f32, name="colB1", tag="colB")
            nc.scalar.activation(sqn1A[:], nl1A[:], AF.Square, accum_out=colA1[:])
            nc.gpsimd.tensor_mul(sqn1B[:], v1B[:], v1B[:])
            nc.vector.reduce_sum(colB1[:], sqn1B[:], axis=AX.X)
            op1 = pp_tile([64, BN], f32, "op1")
            nc.tensor.matmul(op1[:], pks[:, SELS0_O:SELS0_O + 64], v1big[0:16, :],
                             start=True, stop=False)
            nc.tensor.matmul(op1[:], pkb[:, SELKA_O:SELKA_O + 64], sqn1A[:],
                             start=False, stop=False)
            nc.tensor.matmul(op1[:], pks[:, SELKB_O:SELKB_O + 64], sqn1B[:],
                             start=False, stop=True)
            op1s = work.tile([64, BN], bf16, name="op1s", tag="op1s")
            nc.vector.tensor_copy(op1s[:], op1[:])
            ssp1 = pp_tile([3, 1], f32, "ssp1")
            nc.tensor.matmul(ssp1[:], pkf[:, SEL3AF_O:SEL3AF_O + 3].bitcast(f32),
                             colA1[:], start=True, stop=False)
            nc.tensor.matmul(ssp1[:], pkf[0:16, SEL3BF_O:SEL3BF_O + 3].bitcast(f32),
                             colB1[:], start=False, stop=True)
            ssl1 = work.tile([3, 1], f32, name="ssl1", tag="ssl1")
            nc.scalar.copy(ssl1[:], ssp1[:])

            # collective: AllGather of layer-1 SS partials (emitted on Pool
            # queue BEFORE any layer-2 Pool work so it issues promptly)
            cc_in = dram.tile([1, 3], f32)
            cc_out = dram.tile([N_CORES, 3], f32, addr_space="Shared")
            nc.sync.dma_start(cc_in[0, :], ssl1[:, 0])
            nc.sync.dma_start(ss1_d[0, :], ssl1[:, 0])

            # layer-1 output block stored feature-major (host transposes)
            nc.sync.dma_start(out_d[0], op1s[:])
            nc.gpsimd.collective_compute(
                "AllGather", mybir.AluOpType.bypass,
                ins=[cc_in.opt()], outs=[cc_out.opt()],
                replica_groups=[list(range(N_CORES))])

            # ===================== layer-2 head (overlaps collective) ======
            acc2A = {}
            acc2B = psum.tile([80, BN], f32, name="acc2B", tag="B", bufs=1)
            relT2 = {}

            def _post_group2(lam):
                # free acc2A[lam]'s psum slot ASAP: copy + transpose the
                # A-side right after the group's last W1 matmul (interleaves
                # with the next group's product chunks).  The B-side (packed
                # acc2B, tile-granular deps) is handled after all products.
                t = work.tile([128, BN], bf16, name=f"relA2_{lam}",
                              tag=f"relA2_{lam}")
                nc.vector.tensor_copy(t[:], acc2A[lam][:])
                for b in range(B_LOC):
                    rtp = pp_tile([N, 128], bf16, f"rtp2{lam}{b}")
                    nc.tensor.transpose(rtp[:], t[:, b * N:(b + 1) * N], identb)
                    rt = work.tile([N, NV], bf16, name=f"relT2{lam}{b}",
                                   tag=f"relT2{lam}{b}")
                    if b % 2 == 0:
                        nc.vector.tensor_copy(rt[:, 0:128], rtp[:])
                    else:
                        nc.scalar.copy(rt[:, 0:128], rtp[:])
                    relT2[(lam, b)] = rt

            def _alloc2A(lam):
                acc2A[lam] = psum.tile([128, BN], f32, name=f"acc2A{lam}",
                                       tag="A", bufs=3)
                return acc2A[lam]

            emit_products(1, v1big[:], v1B[:], acc2A, acc2B, split=True,
                          post_group=_post_group2, alloc_accA=_alloc2A)
            for lam in GROUP_LAMS:
                bnd = BAND[lam] * 32
                tb = work.tile([16, BN], bf16, name=f"relB2_{lam}",
                               tag=f"relB2_{lam}")
                nc.scalar.copy(tb[:], acc2B[bnd:bnd + 16, :])
                for b in range(B_LOC):
                    rtpB = pp_tile([N, 16], bf16, f"rtpB2{lam}{b}")
                    nc.tensor.transpose(rtpB[:], tb[:, b * N:(b + 1) * N],
                                        identb[0:16, 0:16])
                    if b % 2 == 0:
                        nc.vector.tensor_copy(relT2[(lam, b)][:, 128:144], rtpB[:])
                    else:
                        nc.scalar.copy(relT2[(lam, b)][:, 128:144], rtpB[:])

            # ---------- collective readback + scales ----------
            # (low scheduler priority: these wait ~15us on the collective, and
            # hoisting them into the L2-products window head-of-line blocks
            # the Act/DVE queues there)
            # The tile scheduler orders queues by simulated readiness; the
            # readback ops' only dep is the collective DMA, so it hoists them
            # into the L2-products window where their ~15us wait head-of-line
            # blocks the queues.  Chain a zero-valued operand derived from the
            # last window tile into the first readback op to pin it after the
            # window work in every queue.
            fake0 = work.tile([3, 8], f32, name="fake0", tag="fake0")
            nc.vector.tensor_scalar_mul(fake0[:], relT2[(1, B_LOC - 1)][0:3, 0:8], 0.0)
            ssall = work.tile([3, N_CORES], f32, name="ssall", tag="ssall")
            nc.sync.dma_start(ssall[:], cc_out[:].rearrange("c l -> l c"))
            t1r = work.tile([3, 8], f32, name="t1r", tag="t1r")
            nc.vector.scalar_tensor_tensor(t1r[:], ssall[:], 1.0,
                                           fake0[:], op0=ALU.mult, op1=ALU.add)
            ssg = work.tile([3, 1], f32, name="ssg", tag="ssg")
            nc.vector.reduce_sum(ssg[:], t1r[:], axis=AX.X)
            irs = work.tile([3, 1], f32, name="irs", tag="irs")
            nc.vector.reciprocal(irs[:], ssg[:])
            # s1_l = CVEC_l * sqrt(1/ss_l) = sqrt(irs * CVEC_l^2)
            s1 = work.tile([3, 1], f32, name="s1", tag="s1")
            nc.scalar.activation(s1[:], irs[:], AF.Sqrt,
                                 scale=pkf[0:3, CVEC2_O:CVEC2_O + 1].bitcast(f32))
            vp = pp_tile([128, 3], f32, "vp")
            for lam in range(3):
                nc.tensor.matmul(vp[:, lam:lam + 1],
                                 pkf[0:3, BC_O + lam * 128:BC_O + (lam + 1) * 128].bitcast(f32),
                                 s1[:], start=True, stop=True)
            vecs = work.tile([128, 3], f32, name="vecs", tag="vecs")
            nc.scalar.copy(vecs[:], vp[:])
            # scaled connectivity per group: connS_g = s1_g * connT
            connS = {}
            for gi, lam in enumerate(GROUP_LAMS):
                t = work.tile([N, B_LOC, N], bf16, name=f"connS{lam}",
                              tag=f"connS{lam}")
                if gi == 2:
                    nc.gpsimd.tensor_scalar_mul(t[:], connT[:], vecs[:, lam:lam + 1])
                else:
                    nc.vector.tensor_scalar_mul(t[:], connT[:], vecs[:, lam:lam + 1])
                connS[lam] = t

            # ---------- layer-2 mp: group-combine inside PSUM accum ------
            mpA2 = psum.tile([128, B_LOC, N], f32, name="mpA2", tag="A", bufs=3)
            mpB2 = psum.tile([16, B_LOC, N], f32, name="mpB2", tag="B", bufs=1)
            # A-part matmuls first so the mpF2 copy (z-stage gate) fires as
            # early as possible; B-part overlaps the z A-accumulation
            for b in range(B_LOC):
                for gi, lam in enumerate(GROUP_LAMS):
                    nc.tensor.matmul(mpA2[:, b, :], relT2[(lam, b)][:, 0:128],
                                     connS[lam][:, b, :],
                                     start=(gi == 0), stop=(gi == 2))
            for b in range(B_LOC):
                for gi, lam in enumerate(GROUP_LAMS):
                    nc.tensor.matmul(mpB2[:, b, :], relT2[(lam, b)][:, 128:144],
                                     connS[lam][:, b, :],
                                     start=(gi == 0), stop=(gi == 2))
            mpF2 = work.tile([128, BN], bf16, name="mpF2", tag="mpF2")
            nc.vector.tensor_copy(mpF2[:], mpA2[:].rearrange("f b n -> f (b n)"))
            mpBs2 = work.tile([16, BN], bf16, name="mpBs2", tag="mpBs2")
            nc.scalar.copy(mpBs2[:], mpB2[:].rearrange("f b n -> f (b n)"))

            nl2A, nl2B = emit_quad(1, mpA2, mpB2, mpF2, mpBs2)

            # layer-2 outputs (raw)
            v2s0 = work.tile([16, BN], bf16, name="v2s0", tag="v2s0")
            nc.vector.tensor_copy(v2s0[:], nl2A[0:16, :])
            sqn2A = work.tile([128, BN], bf16, name="sqn2A", tag="sqn2A")
            sqn2B = work.tile([16, BN], bf16, name="sqn2B", tag="sqn2B")
            colA2 = work.tile([128, 1], f32, name="colA2", tag="colA2")
            colB2 = work.tile([16, 1], f32, name="colB2", tag="colB2")
            nc.scalar.activation(sqn2A[:], nl2A[:], AF.Square, accum_out=colA2[:])
            v2B = work.tile([16, BN], bf16, name="v2B", tag="v2B")
            nc.vector.tensor_copy(v2B[:], nl2B[:])
            nc.vector.tensor_mul(sqn2B[:], v2B[:], v2B[:])
            nc.vector.reduce_sum(colB2[:], sqn2B[:], axis=AX.X)
            op2 = pp_tile([64, BN], f32, "op2")
            nc.tensor.matmul(op2[:], pks[:, SELS0_O:SELS0_O + 64], v2s0[:],
                             start=True, stop=False)
            nc.tensor.matmul(op2[:], pkb[:, SELKA_O:SELKA_O + 64], sqn2A[:],
                             start=False, stop=False)
            nc.tensor.matmul(op2[:], pks[:, SELKB_O:SELKB_O + 64], sqn2B[:],
                             start=False, stop=True)
            outl2 = work.tile([64, BN], bf16, name="outl2", tag="outl2")
            nc.vector.tensor_copy(outl2[:], op2[:])
            ssp2 = pp_tile([3, 1], f32, "ssp2")
            nc.tensor.matmul(ssp2[:], pkf[:, SEL3AF_O:SEL3AF_O + 3].bitcast(f32),
                             colA2[:], start=True, stop=False)
            nc.tensor.matmul(ssp2[:], pkf[0:16, SEL3BF_O:SEL3BF_O + 3].bitcast(f32),
                             colB2[:], start=False, stop=True)
            ssl2 = work.tile([3, 1], f32, name="ssl2", tag="ssl2")
            nc.scalar.copy(ssl2[:], ssp2[:])
            nc.sync.dma_start(ss2_d[0, :], ssl2[:, 0])

            # layer-2 output block stored feature-major (host transposes)
            nc.sync.dma_start(out_d[1], outl2[:])

    if SPLIT_WAITS:
        _split_waits(nc, mybir)
    return nc


def _split_waits(nc, mybir, maxw=1):
    """This container's walrus rejects instructions carrying more than one
    semaphore wait; spill extra waits onto same-engine NoOps placed just
    before the instruction."""
    k = [0]
    for f in nc.m.functions:
        for bb in f.blocks:
            newl = []
            for ins in bb.instructions:
                si = ins.sync_info
                if si is not None and len(si.on_wait) > maxw:
                    waits = list(si.on_wait)
                    for w in waits[:-maxw]:
                        k[0] += 1
                        nop = mybir.InstDrain(name=f"wsplit_{k[0]}", ins=[], outs=[])
                        nop.engine = ins.engine
                        nop.sync_info = mybir.SyncInfo(on_wait=[w], on_update=[])
                        newl.append(nop)
                    ins.sync_info = mybir.SyncInfo(on_wait=waits[-maxw:],
                                                  on_update=list(si.on_update))
                newl.append(ins)
            bb.instructions = newl


def _get_program():
    global _BUILT
    if _BUILT is None:
        _BUILT = build_program()
    return _BUILT


def _make_const_inputs(Wr, Wn):
    import ml_dtypes
    bf = ml_dtypes.bfloat16

    def _pad_chunks(M, chunks):
        out = np.zeros((len(chunks) * 128, M.shape[1]), np.float32)
        for ci, (r0, nr) in enumerate(chunks):
            out[ci * 128:ci * 128 + nr] = M[r0:r0 + nr]
        return out

    w1c = [(r0, nr) for (_lam, r0, nr) in PCHUNKS]
    w1s, lins = [], []
    for ly in range(LAYERS):
        W1, W1L = build_W1({l: np.asarray(Wr[l][ly]) for l in range(3)})
        w1s.append(_pad_chunks(W1, w1c))
        lins.append(W1L)
    w1 = np.stack(w1s)                                 # [2, 1152, 144]
    w3 = np.stack([_pad_chunks(build_W3({l: np.asarray(Wn[l][ly]) for l in range(3)}), W3CH)
                   for ly in range(LAYERS)])           # [2, 768, 144]
    # dram layout [128, L*12, NV] / [128, L*6, NV]: p-major tiling
    w1t = w1.reshape(LAYERS, 9, 128, NV).transpose(2, 0, 1, 3).reshape(128, LAYERS * 9, NV)
    w3t = w3.reshape(LAYERS, 6, 128, NV).transpose(2, 0, 1, 3).reshape(128, LAYERS * 6, NV)
    # j=0 linear packs: slot 0 = layer-1 combined; 1..3 = layer-2 per group
    lin4 = np.zeros((4, NV, NV), np.float32)
    lin4[0] = lins[0]
    grows = {0: (0, 16), 2: (16, 96), 1: (96, 144)}
    for gi, lam in enumerate(GROUP_LAMS):
        a, b = grows[lam]
        lin4[1 + gi, a:b, :] = lins[1][a:b, :]
    linm = lin4[:, 0:128, :].transpose(1, 0, 2)        # [128, 4, NV]
    linb = lin4[:, 128:144, :].transpose(1, 0, 2)      # [16, 4, NV]

    L2 = build_L2()
    selk = build_SELK()
    sel3 = build_SEL3()
    pkf = np.zeros((128, F_COLS), np.float32)
    pkf[:, IDENT_O:IDENT_O + 128] = np.eye(128, dtype=np.float32)
    pkf[0:3, BC_O:BC_O + 384] = build_BC()
    pkf[0:3, CVEC2_O:CVEC2_O + 1] = (C / np.array([[1.0], [3.0], [5.0]], np.float32)) ** 2
    pkf[0:128, SEL3AF_O:SEL3AF_O + 3] = sel3[0:128]
    pkf[0:16, SEL3BF_O:SEL3BF_O + 3] = sel3[128:144]
    pkb = np.zeros((128, B_COLS), np.float32)
    pkb[:, IDENTB_O:IDENTB_O + 128] = np.eye(128, dtype=np.float32)
    pkb[0:128, L2A_O:L2A_O + 512] = L2[0:128]
    pkb[0:128, SELKA_O:SELKA_O + 64] = selk[0:128]
    pkb[0:128, SEL3A_O:SEL3A_O + 3] = sel3[0:128]
    pkb[0:9, MMAT_O:MMAT_O + 16] = build_MMAT()
    pks = np.zeros((16, S_COLS), np.float32)
    pks[:, E9_O:E9_O + 12 * 128] = E9M
    pks[:, L2B_O:L2B_O + 512] = L2[128:144]
    pks[:, SELKB_O:SELKB_O + 64] = selk[128:144]
    pks[:, SELS0_O:SELS0_O + 64] = build_SELS0()
    pks[:, SEL3B_O:SEL3B_O + 3] = sel3[128:144]
    return {
        "pkf": pkf,
        "pkb": pkb.astype(bf),
        "pks": pks.astype(bf),
        "vsa": VSA[:, 0:9 * 128].astype(bf),
        "vsb": VSB[:, 0:9 * 128].astype(bf),
        "w1": w1t.astype(bf),
        "w3": w3t.astype(bf),
        "linm": linm.astype(bf),
        "linb": linb.astype(bf),
    }


def kernel(v0, v1, v2, rel_pos, norms, Wr0, Wr1, Wr2, Wn0, Wn1, Wn2):
    import ml_dtypes
    bf = ml_dtypes.bfloat16
    v0, v1, v2 = np.asarray(v0), np.asarray(v1), np.asarray(v2)
    rel_pos, norms = np.asarray(rel_pos), np.asarray(norms)
    consts = _make_const_inputs({0: Wr0, 1: Wr1, 2: Wr2},
                                {0: Wn0, 1: Wn1, 2: Wn2})
    vfm = np.concatenate([v0.reshape(B_GLOB * N, 16),
                          v2.reshape(B_GLOB * N, 80),
                          v1.reshape(B_GLOB * N, 48)], axis=-1).T  # [144, B*N]
    in_maps = []
    for cc in range(N_CORES):
        sl = slice(cc * B_LOC, (cc + 1) * B_LOC)
        m = dict(consts)
        vcore = vfm[:, cc * B_LOC * N:(cc + 1) * B_LOC * N]
        m["vfa"] = np.ascontiguousarray(vcore[0:128]).astype(bf)
        m["vfb"] = np.ascontiguousarray(vcore[128:144]).astype(bf)
        m["rp"] = np.ascontiguousarray(rel_pos[sl]).astype(bf)
        m["norms"] = np.ascontiguousarray(norms[sl])
        in_maps.append(m)

    from concourse.bass_utils import run_bass_kernel_spmd
    nc = _get_program()
    res = run_bass_kernel_spmd(nc, in_maps, list(range(N_CORES)))
    # per-core out is [2, 64, B_LOC*N] feature-major; -> [B_LOC, N, 128]
    outs = [np.asarray(res.results[cc]["out"], np.float32)
            .reshape(2, 64, B_LOC, N).transpose(2, 3, 0, 1).reshape(B_LOC, N, 128)
            for cc in range(N_CORES)]
    ss1 = sum(res.results[cc]["ss1"][0] for cc in range(N_CORES))
    ss2 = sum(res.results[cc]["ss2"][0] for cc in range(N_CORES))
    out = np.concatenate(outs, axis=0)  # [32, 128, 128]
    scale1 = C / (np.array([1.0, 3.0, 5.0], np.float32) * np.sqrt(ss1))
    scale2 = C / (np.array([1.0, 3.0, 5.0], np.float32) * np.sqrt(ss2))
    fin = np.ones(128, np.float32)
    fin[0:16] = scale1[0]
    fin[16:32] = scale1[0] ** 2
    fin[32:48] = scale1[1] ** 2
    fin[48:64] = scale1[2] ** 2
    fin[64:80] = scale2[0]
    fin[80:96] = scale2[0] ** 2
    fin[96:112] = scale2[1] ** 2
    fin[112:128] = scale2[2] ** 2
    return (out * fin[None, None, :]).astype(np.float32)
